# revision 42
# baseline (speedup 1.0000x reference)
"""Trainium2 Bass kernel for nn_MemTransformerLM (Transformer-XL rel-attention).

v3 on top of v2: rT host-folded (pos_emb @ Wr is weight-only), k/v projected
before q so the AllGather hides behind q-proj + the E pass, weight/KV-gather
DMAs moved to the SWDGE (gpsimd) queue so the sync queue never stalls on the
collective, E matmuls issued in head pairs (row-group concurrency), minimal
P-buffer init.

v2: causally-truncated, load-balanced attention. Every core runs the same
program (SPMD single NEFF) over three 128-row "slots" whose row content is
core-dependent:
  - big   slot: rows [1952-128*pid, 2080-128*pid), scores over j in [0, 2080)
  - small slot: rows [  32+128*pid,  160+128*pid), scores over j in [0, 1056)
  - micro slot: rows [0, 32) (replicated on all cores; core 0's output used),
                scores over j in [0, 128)
Per-core score columns: 2080+1056+128 = 3264 vs 3*2080 = 6240 for the naive
contiguous-span split -- ~1.9x less attention work, perfectly balanced.

The Transformer-XL _rel_shift uses the padded-DRAM-pitch trick: per (head,
slot) a private DRAM buffer [129, 2081]; E rows are written at row pitch
T+1 = 2081 behind a zero column and read back at row stride T with a
rank-dependent offset, reproducing the reference's wrap semantics (including
the "garbage" upper-triangle values inside the two mem-token corners).
Private per-(head,slot) buffers keep the write->read round trips of
different heads independent so the pipeline overlaps across heads.

Masking: big/micro slots use host-precomputed additive masks. The small slot
needs no mask at all: its masked positions (j > i) always wrap into the
never-written low-column region of its P buffer, which is poisoned once with
-1e30 at kernel start.
"""

import os

import numpy as np
import ml_dtypes

import concourse.bass as bass
import concourse.mybir as mybir
import concourse.tile as tile
from concourse import bacc
from concourse.bass import ds
from concourse.bass_utils import run_bass_kernel_spmd
from concourse.masks import make_identity

BF16 = ml_dtypes.bfloat16
DT = mybir.dt
AF = mybir.ActivationFunctionType
ALU = mybir.AluOpType

N_LAYER = 4
N_HEAD = 8
D_HEAD = 64
D_MODEL = 512
D_INNER = 2048
NMT = 16
T = 2048 + 2 * NMT      # 2080
N_CORES = 8
NEG = -1e30
SCALE = 1.0 / np.sqrt(D_HEAD)
HP = 4                  # 128-partition tiles over D_MODEL

# slots: (q-col base, score width, padded prob width, n j-tiles, E k-window lo)
W_BIG = 2080
W_SMALL = 1056
W_MICRO = 128
PAD_BIG = 17 * 128      # 2176
PAD_SMALL = 9 * 128     # 1152
CH_BIG = [512, 512, 512, 512, 32]
CH_SMALL = [512, 512, 32]
PROWS = 129             # 128 rows + 1 wrap-margin row per (head, slot) buffer
PITCH = T + 1           # 2081
POISON_W = 1025         # small-slot poison columns [0, POISON_W)
MASKB_W = 1152          # big-slot mask covers j in [W_BIG-MASKB_W, W_BIG) only

NK_TOK = 288            # tokens contributed per core to the allgather (128+128+32)
AG_N = D_MODEL * NK_TOK + NK_TOK * D_MODEL  # kT part + v part, elements


def _row_bases(m):
    return 1952 - 128 * m, 32 + 128 * m  # big, small global row starts


def _host_prep(inputs):
    word_emb = np.asarray(inputs["word_emb"], np.float32)
    mem_tokens = np.asarray(inputs["mem_tokens"], np.float32)
    w = np.transpose(word_emb, (1, 0, 2))[:, 0, :]
    mem = mem_tokens[:, 0, :]
    w_full = np.concatenate([mem, w, mem], axis=0)          # [T, 512]

    inv_freq = 1.0 / (10000.0 ** (np.arange(0, D_MODEL, 2, dtype=np.float32) / D_MODEL))
    pos_seq = np.arange(T - 1, -1, -1.0, dtype=np.float32)
    sinusoid = pos_seq[:, None] * inv_freq[None, :]
    pos_emb = np.concatenate([np.sin(sinusoid), np.cos(sinusoid)], axis=-1)

    wqkv = np.asarray(inputs["Wqkv"], np.float32).astype(BF16)
    # r projection depends only on weights + the fixed sinusoid table:
    # fold it on the host, ship rT = (pos_emb @ Wr[l]).T per layer.
    wr_f32 = np.asarray(inputs["Wr"], np.float32)
    rT = np.ascontiguousarray(
        np.stack([(pos_emb @ wr_f32[l]).T for l in range(N_LAYER)])
    ).astype(BF16)                                          # [L, 512, T]
    wo = np.asarray(inputs["Wo"], np.float32).astype(BF16)
    w1 = np.asarray(inputs["ffn_W1"], np.float32).astype(BF16)
    w2 = np.asarray(inputs["ffn_W2"], np.float32).astype(BF16)
    # fold the 1/sqrt(d) score scale into q (and its biases)
    rwb = np.ascontiguousarray(
        (np.asarray(inputs["r_w_bias"], np.float32) * SCALE).reshape(-1, 1))
    rrb = np.ascontiguousarray(
        (np.asarray(inputs["r_r_bias"], np.float32) * SCALE).reshape(-1, 1))

    # reference mask, True = masked
    M = np.triu(np.ones((T, T), dtype=bool), k=1)
    M[:NMT, :NMT] = False
    M[-NMT:, -NMT:] = False

    mask_micro = np.zeros((128, W_MICRO), np.float32)
    mask_micro[:32, :] = np.where(M[:32, :W_MICRO], NEG, 0.0)

    per_core = []
    for rank in range(N_CORES):
        rb, rs = _row_bases(rank)
        w0 = np.zeros((384, D_MODEL), np.float32)
        w0[0:128] = w_full[rb : rb + 128]
        w0[128:256] = w_full[rs : rs + 128]
        w0[256:288] = w_full[0:32]
        mask_big = np.where(M[rb : rb + 128, W_BIG - MASKB_W :], NEG, 0.0).astype(BF16)
        assert not M[rb : rb + 128, : W_BIG - MASKB_W].any()
        per_core.append(
            {
                "w0": w0,
                "rT": rT,
                "wqkv": wqkv,
                "wo": wo,
                "w1": w1,
                "w2": w2,
                "rwb": rwb,
                "rrb": rrb,
                "mask_big": np.ascontiguousarray(mask_big),
                "mask_micro": np.ascontiguousarray(mask_micro.astype(BF16)),
            }
        )

    gb = np.ascontiguousarray(
        np.stack(
            [
                np.broadcast_to(np.asarray(inputs["ln1_scale"], np.float32)[:, None, :],
                                (N_LAYER, 128, D_MODEL)),
                np.broadcast_to(np.asarray(inputs["ln1_bias"], np.float32)[:, None, :],
                                (N_LAYER, 128, D_MODEL)),
                np.broadcast_to(np.asarray(inputs["ln2_scale"], np.float32)[:, None, :],
                                (N_LAYER, 128, D_MODEL)),
                np.broadcast_to(np.asarray(inputs["ln2_bias"], np.float32)[:, None, :],
                                (N_LAYER, 128, D_MODEL)),
            ],
            axis=2,
        ).astype(np.float32)
    )
    b1col = np.ascontiguousarray(
        np.asarray(inputs["ffn_b1"], np.float32).reshape(N_LAYER, D_INNER, 1))
    b2bc = np.ascontiguousarray(
        np.broadcast_to(np.asarray(inputs["ffn_b2"], np.float32)[:, None, :],
                        (N_LAYER, 128, D_MODEL)).copy())
    for pc in per_core:
        pc["gb"] = gb
        pc["b1col"] = b1col
        pc["b2bc"] = b2bc
    return per_core


def _layernorm(nc, sm, out_ap, x, g, b, eps):
    f32 = DT.float32
    stats = sm.tile([128, 6], f32, tag="lnst")
    mv = sm.tile([128, 2], f32, tag="lnmv")
    nc.vector.bn_stats(stats[:], x[:])
    nc.vector.bn_aggr(mv[:], stats[:])
    std = sm.tile([128, 1], f32, tag="lnstd")
    nc.scalar.activation(std[:], mv[:, 1:2], AF.Sqrt, bias=eps, scale=1.0)
    rstd = sm.tile([128, 1], f32, tag="lnrstd")
    nc.vector.reciprocal(rstd[:], std[:])
    if g is None:
        nc.vector.tensor_scalar(
            out=out_ap, in0=x[:], scalar1=mv[:, 0:1], scalar2=rstd[:],
            op0=ALU.subtract, op1=ALU.mult,
        )
    else:
        xn = sm.tile([128, D_MODEL], f32, tag="lnxn")
        nc.vector.tensor_scalar(
            out=xn[:], in0=x[:], scalar1=mv[:, 0:1], scalar2=rstd[:],
            op0=ALU.subtract, op1=ALU.mult,
        )
        nc.vector.tensor_tensor(xn[:], xn[:], g, ALU.mult)
        nc.vector.tensor_tensor(out_ap, xn[:], b, ALU.add)


def _build(trivial_gb=True, trivial_b=True):
    nc = bacc.Bacc("TRN2", num_devices=N_CORES, dynamic_dma_scratch_size=4096)
    f32, bf16 = DT.float32, DT.bfloat16

    w0_t = nc.dram_tensor("w0", [384, D_MODEL], f32, kind="ExternalInput")
    rT_t = nc.dram_tensor("rT", [N_LAYER, D_MODEL, T], bf16, kind="ExternalInput")
    wqkv_t = nc.dram_tensor("wqkv", [N_LAYER, D_MODEL, 3 * D_MODEL], bf16, kind="ExternalInput")
    wo_t = nc.dram_tensor("wo", [N_LAYER, D_MODEL, D_MODEL], bf16, kind="ExternalInput")
    w1_t = nc.dram_tensor("w1", [N_LAYER, D_MODEL, D_INNER], bf16, kind="ExternalInput")
    w2_t = nc.dram_tensor("w2", [N_LAYER, D_INNER, D_MODEL], bf16, kind="ExternalInput")
    rwb_t = nc.dram_tensor("rwb", [D_MODEL, 1], f32, kind="ExternalInput")
    rrb_t = nc.dram_tensor("rrb", [D_MODEL, 1], f32, kind="ExternalInput")
    maskb_t = nc.dram_tensor("mask_big", [128, MASKB_W], bf16, kind="ExternalInput")
    maskm_t = nc.dram_tensor("mask_micro", [128, W_MICRO], bf16, kind="ExternalInput")
    gb_t = None if trivial_gb else nc.dram_tensor(
        "gb", [N_LAYER, 128, 4, D_MODEL], f32, kind="ExternalInput")
    b1_t = b2_t = None
    if not trivial_b:
        b1_t = nc.dram_tensor("b1col", [N_LAYER, D_INNER, 1], f32, kind="ExternalInput")
        b2_t = nc.dram_tensor("b2bc", [N_LAYER, 128, D_MODEL], f32, kind="ExternalInput")
    out_t = nc.dram_tensor("wout", [384, D_MODEL], f32, kind="ExternalOutput")

    # per-(head-pair, slot) rel-shift buffers, [2, PROWS, PITCH] flat so one
    # DMA serves both heads of a pair
    PP = PROWS * PITCH
    p_big = [nc.dram_tensor(f"pb{q}", [2 * PP], bf16, kind="Internal")
             for q in range(N_HEAD // 2)]
    p_small = [nc.dram_tensor(f"psm{q}", [2 * PP], bf16, kind="Internal")
               for q in range(N_HEAD // 2)]
    p_micro = [nc.dram_tensor(f"pmi{q}", [2 * PP], bf16, kind="Internal")
               for q in range(N_HEAD // 2)]

    ag_in = nc.dram_tensor("ag_in", [AG_N], bf16, kind="Internal")
    ag_out = nc.dram_tensor("ag_out", [N_CORES, AG_N], bf16, kind="Internal",
                            addr_space="Shared")
    rg = [list(range(N_CORES))]

    kv_off = D_MODEL * NK_TOK
    agin_k = ag_in[0:kv_off].rearrange("(a b) -> a b", b=NK_TOK)      # [512, 288]
    agin_v = ag_in[kv_off:].rearrange("(a b) -> a b", b=D_MODEL)      # [288, 512]

    with tile.TileContext(nc, num_cores=N_CORES) as tc:
        pid = nc.sync.partition_id()
        with (
            tc.tile_pool(name="const", bufs=1) as constp,
            tc.tile_pool(name="pers", bufs=1) as pers,
            tc.tile_pool(name="wts", bufs=1) as wts,
            tc.tile_pool(name="kv", bufs=1) as kvp,
            tc.tile_pool(name="mid", bufs=2) as mid,
            tc.tile_pool(name="epool", bufs=2) as epool,
            tc.tile_pool(name="bdp", bufs=2) as bdp,
            tc.tile_pool(name="probp", bufs=1) as probp,
            tc.tile_pool(name="ptp", bufs=1) as ptp,
            tc.tile_pool(name="sm", bufs=2) as sm,
            tc.tile_pool(name="ps", bufs=3, space="PSUM") as ps,
            tc.tile_pool(name="pspv", bufs=2, space="PSUM") as pspv,
            tc.tile_pool(name="psff", bufs=1, space="PSUM") as psff,
        ):
            ident = constp.tile([128, 128], f32)
            make_identity(nc, ident[:])
            identb = constp.tile([128, 128], bf16)
            nc.vector.tensor_copy(identb[:], ident[:])

            # ---- init P buffers (minimal read-coverage init, verified by
            # symbolic read-region check; see sim_check.py) ----
            zrow = epool.tile([128, W_BIG], bf16, tag="esb")
            nc.vector.memset(zrow[:], 0.0)
            for q in range(N_HEAD // 2):
                for e in (0, 1):
                    # big: zero-pad col 0 (rows 0-128) + wrap row 128
                    v2 = p_big[q][e * PP : (e + 1) * PP].rearrange(
                        "(r c) -> r c", c=PITCH)
                    nc.sync.dma_start(v2[0:128, 0:1], zrow[:, 0:1])
                    nc.sync.dma_start(v2[128:129, 0:W_BIG], zrow[:1, :])
                    nc.sync.dma_start(v2[128:129, W_BIG:PITCH], zrow[:1, 0:1])
                    # micro: rows 0-33 wide + rows 34-128 low cols
                    v2 = p_micro[q][e * PP : (e + 1) * PP].rearrange(
                        "(r c) -> r c", c=PITCH)
                    nc.sync.dma_start(v2[0:34, 0:1953], zrow[:34, :1953])
                    nc.sync.dma_start(v2[34:129, 0:128], zrow[:95, :128])
            prow = epool.tile([128, W_BIG], bf16, tag="esb")
            nc.vector.memset(prow[:], NEG)
            for q in range(N_HEAD // 2):
                for e in (0, 1):
                    v2 = p_small[q][e * PP : (e + 1) * PP].rearrange(
                        "(r c) -> r c", c=PITCH)
                    nc.sync.dma_start(v2[0:128, 0:POISON_W], prow[:, :POISON_W])
                    nc.sync.dma_start(v2[128:129, 0:POISON_W], prow[:1, :POISON_W])

            rwb_sb = pers.tile([128, HP], f32)
            rrb_sb = pers.tile([128, HP], f32)
            for d in range(HP):
                nc.sync.dma_start(rwb_sb[:, d : d + 1], rwb_t[d * 128 : (d + 1) * 128, :])
                nc.sync.dma_start(rrb_sb[:, d : d + 1], rrb_t[d * 128 : (d + 1) * 128, :])

            eps_sb = pers.tile([128, 1], f32)
            nc.vector.memset(eps_sb[:], 1e-5)
            maskb_sb = pers.tile([128, MASKB_W], bf16)
            nc.sync.dma_start(maskb_sb[:], maskb_t[:])
            maskm_sb = pers.tile([128, W_MICRO], bf16)
            nc.sync.dma_start(maskm_sb[:], maskm_t[:])
            w_sb = pers.tile([128, 3, D_MODEL], f32)
            for qt in range(3):
                nc.sync.dma_start(w_sb[:, qt, :], w0_t[qt * 128 : (qt + 1) * 128, :])

            for l in range(N_LAYER):
                # ---- layer weights + host-folded rT (SWDGE queue, off the
                # sync engine so they overlap the attention DMA pipeline) ----
                wqkv_sb = wts.tile([128, HP, 3 * D_MODEL], bf16, tag="wqkv")
                wo_sb = wts.tile([128, HP, D_MODEL], bf16, tag="wol")
                w1_sb = wts.tile([128, HP, D_INNER], bf16, tag="w1l")
                w2_sb = wts.tile([128, 16, D_MODEL], bf16, tag="w2l")
                rT_sb = wts.tile([128, HP, T], bf16, tag="rT")
                # one consolidated DMA per weight tensor: src dims (p, d, c)
                nc.sync.dma_start(
                    wqkv_sb[:, :, :],
                    wqkv_t[l].rearrange("(d p) c -> p d c", p=128),
                )
                nc.sync.dma_start(
                    rT_sb[:, :, :], rT_t[l].rearrange("(d p) c -> p d c", p=128)
                )
                nc.sync.dma_start(
                    wo_sb[:, :, :], wo_t[l].rearrange("(d p) c -> p d c", p=128)
                )
                nc.sync.dma_start(
                    w1_sb[:, :, :], w1_t[l].rearrange("(d p) c -> p d c", p=128)
                )
                nc.sync.dma_start(
                    w2_sb[:, :, :], w2_t[l].rearrange("(d p) c -> p d c", p=128)
                )
                gb_sb = None
                if not trivial_gb:
                    gb_sb = wts.tile([128, 4, D_MODEL], f32, tag="gbl")
                    nc.sync.dma_start(gb_sb[:], gb_t[l])
                b1_sb = b2_sb = None
                if not trivial_b:
                    b1_sb = wts.tile([128, 16], f32, tag="b1l")
                    for d in range(16):
                        nc.sync.dma_start(b1_sb[:, d : d + 1], b1_t[l, d * 128 : (d + 1) * 128, :])
                    b2_sb = wts.tile([128, D_MODEL], f32, tag="b2l")
                    nc.sync.dma_start(b2_sb[:], b2_t[l])

                # ---- transpose residual -> wT bf16 ----
                wT_sb = wts.tile([128, HP, 384], bf16, tag="wT")
                for qt in range(3):
                    for d in range(HP):
                        pt = ps.tile([128, 512], f32, tag="pp")
                        nc.tensor.transpose(
                            pt[:, :128], w_sb[:, qt, d * 128 : (d + 1) * 128], ident[:]
                        )
                        nc.scalar.copy(wT_sb[:, d, qt * 128 : (qt + 1) * 128], pt[:, :128])

                # ---- k/v projections first so the AllGather can launch
                # early and hide behind q-proj + the whole E pass ----
                kT_own = wts.tile([128, HP, NK_TOK], bf16, tag="kTown")
                for hp in range(HP):
                    pk = ps.tile([128, 512], f32, tag="pp")
                    for d in range(HP):
                        nc.tensor.matmul(
                            pk[:, :384],
                            wqkv_sb[:, d, D_MODEL + hp * 128 : D_MODEL + hp * 128 + 128],
                            wT_sb[:, d, :],
                            start=(d == 0), stop=(d == HP - 1),
                        )
                    nc.scalar.copy(kT_own[:, hp, :], pk[:, :NK_TOK])
                nc.sync.dma_start(
                    agin_k.rearrange("(hp p) c -> p hp c", p=128), kT_own[:, :, :]
                )
                v_own = wts.tile([128, 3, D_MODEL], bf16, tag="vown")
                for qt in range(3):
                    pv = ps.tile([128, 512], f32, tag="pp")
                    for d in range(HP):
                        nc.tensor.matmul(
                            pv[:],
                            wT_sb[:, d, qt * 128 : (qt + 1) * 128],
                            wqkv_sb[:, d, 2 * D_MODEL :],
                            start=(d == 0), stop=(d == HP - 1),
                        )
                    nc.vector.tensor_copy(v_own[:, qt, :], pv[:])
                    rows = 32 if qt == 2 else 128
                    nc.sync.dma_start(
                        agin_v[qt * 128 : qt * 128 + rows, :], v_own[:rows, qt, :]
                    )

                nc.gpsimd.collective_compute(
                    "AllGather", ALU.bypass, replica_groups=rg,
                    ins=[ag_in[:]], outs=[ag_out[:]],
                )

                # ---- q projection (pre-scaled by 1/sqrt(d)) ----
                qwT = wts.tile([128, HP, 384], bf16, tag="qwT")
                qrT = wts.tile([128, HP, 384], bf16, tag="qrT")
                for hp in range(HP):
                    pq = ps.tile([128, 512], f32, tag="pp")
                    for d in range(HP):
                        nc.tensor.matmul(
                            pq[:, :384],
                            wqkv_sb[:, d, hp * 128 : hp * 128 + 128],
                            wT_sb[:, d, :],
                            start=(d == 0), stop=(d == HP - 1),
                        )
                    nc.scalar.activation(
                        qwT[:, hp, :], pq[:, :384], AF.Identity,
                        bias=rwb_sb[:, hp : hp + 1], scale=float(SCALE),
                    )
                    nc.scalar.activation(
                        qrT[:, hp, :], pq[:, :384], AF.Identity,
                        bias=rrb_sb[:, hp : hp + 1], scale=float(SCALE),
                    )

                # ---- attention ----
                # slot parameters: (qcol0, width, padded width, chunks,
                #                   k-window lo, p-tensor list, probT tile0, n tiles)
                def slot_params(h, si):
                    q = h // 2
                    if si == 0:
                        return (0, W_BIG, PAD_BIG, CH_BIG, 0, p_big[q],
                                128 + pid * 128, 0, 17)
                    if si == 1:
                        return (128, W_SMALL, PAD_SMALL, CH_SMALL, T - W_SMALL,
                                p_small[q], 2048 - pid * 128, 17, 9)
                    return (256, W_MICRO, W_MICRO, [128], T - W_MICRO,
                            p_micro[q], 2080, 26, 1)

                # E computation for a PAIR of heads -> DRAM. The pair's lhsT
                # base partitions are 0 / 64, so bass auto-derives row-group
                # tile positions and the two K=64 matmuls run concurrently in
                # the PE array. PSUM evacuation alternates scalar/vector.
                def e_pass_pair(h0):
                    hp = h0 // 2
                    for si in range(3):
                        qc0, wj, wpad, chs, klo, pt_t, off, jt0, njt = slot_params(h0, si)
                        # pair view [row, e, col] so one DMA writes both heads
                        p3d = pt_t.rearrange("(e r c) -> r e c", e=2, c=PITCH)
                        qsl = slice(qc0, qc0 + 128)
                        e_sb = epool.tile([128, 2, W_BIG], bf16, tag="esb", name="e_sb")
                        cpos = 0
                        for cw in chs:
                            pe = [
                                ps.tile([128, 512], f32, tag="pp", name="pe0"),
                                ps.tile([128, 512], f32, tag="pp", name="pe1"),
                            ]
                            for e in (0, 1):
                                nc.tensor.matmul(
                                    pe[e][:, :cw],
                                    qrT[e * 64 : e * 64 + 64, hp, qsl],
                                    rT_sb[e * 64 : e * 64 + 64, hp, klo + cpos : klo + cpos + cw],
                                    start=True, stop=True,
                                )
                            nc.scalar.copy(e_sb[:, 0, cpos : cpos + cw], pe[0][:, :cw])
                            nc.vector.tensor_copy(e_sb[:, 1, cpos : cpos + cw], pe[1][:, :cw])
                            cpos += cw
                        if si == 2:
                            # corner-garbage columns k in [0, 16)
                            pe = [
                                ps.tile([128, 512], f32, tag="pp", name="pec0"),
                                ps.tile([128, 512], f32, tag="pp", name="pec1"),
                            ]
                            for e in (0, 1):
                                nc.tensor.matmul(
                                    pe[e][:, :16],
                                    qrT[e * 64 : e * 64 + 64, hp, qsl],
                                    rT_sb[e * 64 : e * 64 + 64, hp, 0:16],
                                    start=True, stop=True,
                                )
                            ec = mid.tile([128, 2, 16], bf16, tag="ecrn")
                            nc.scalar.copy(ec[:, 0, :], pe[0][:, :16])
                            nc.vector.tensor_copy(ec[:, 1, :], pe[1][:, :16])
                            nc.sync.dma_start(p3d[0:128, :, 1:17], ec[:, :, :])
                        nc.sync.dma_start(
                            p3d[0:128, :, 1 + klo : 1 + klo + wj], e_sb[:, :, :wj]
                        )

                # pass 2: shifted read-back, scores, softmax, PV.
                # BD reads are prefetched one head ahead so they are never
                # queued behind the probT transposes on the sync queue.
                attnT = wts.tile([128, HP, 384], bf16, tag="attnT")
                BD_TAGS = ["bdb", "bds", "bdm"]
                BD_W = [W_BIG, W_SMALL, W_MICRO]

                def issue_reads_pair(h0):
                    bds = []
                    for si in range(3):
                        sp = slot_params(h0, si)
                        wj, off, pt_t = sp[1], sp[6], sp[5]
                        bd_sb = bdp.tile([128, 2, BD_W[si]], bf16, tag=BD_TAGS[si])
                        src_ap = pt_t.rearrange("(e x) -> e x", e=2)[
                            :, ds(off, 128 * T)
                        ].rearrange("e (p j) -> p e j", j=T)
                        nc.sync.dma_start(bd_sb[:, :, :wj], src_ap[:, :, :wj])
                        bds.append(bd_sb)
                    return bds

                ppv_prev = [None]

                def flush_attnT():
                    if ppv_prev[0] is not None:
                        php, ptile = ppv_prev[0]
                        nc.vector.tensor_copy(attnT[:, php, :], ptile[:])
                        ppv_prev[0] = None

                def pass2_pair(h0, bds, first):
                    hp = h0 // 2
                    ppv = pspv.tile([128, 384], f32, tag="ppv", name="ppv")
                    for si in range(3):
                        qc0, wj, wpad, chs, klo, _, off, jt0, njt = slot_params(h0, si)
                        qsl = slice(qc0, qc0 + 128)
                        bd_sb = bds[si]

                        # scores: AC + BD (+ mask for big/micro; small is
                        # handled entirely by the poison region)
                        for e in (0, 1):
                            if si == 0:
                                nc.vector.tensor_tensor(
                                    bd_sb[:, e, W_BIG - MASKB_W :],
                                    bd_sb[:, e, W_BIG - MASKB_W :], maskb_sb[:], ALU.add,
                                )
                            elif si == 2:
                                nc.vector.tensor_tensor(
                                    bd_sb[:, e, :wj], bd_sb[:, e, :wj],
                                    maskm_sb[:, :wj], ALU.add,
                                )
                        prob = probp.tile([128, 2, wpad], bf16, tag=f"prob{si}")
                        probT = ptp.tile([128, 2, njt, 128], bf16, tag=f"probT{si}")
                        if wpad > wj and first:
                            # pad cols stay zero across heads/layers
                            # (x*rden keeps 0 at 0)
                            nc.vector.memset(prob[:, 0, wj:wpad], 0.0)
                            nc.vector.memset(prob[:, 1, wj:wpad], 0.0)
                        cpos = 0
                        dn1 = []
                        for cw in chs:
                            jsl = slice(cpos, cpos + cw)
                            pa = [
                                ps.tile([128, 512], f32, tag="pp", name="pa0"),
                                ps.tile([128, 512], f32, tag="pp", name="pa1"),
                            ]
                            # e=0: AC on PE, +BD on DVE, exp later (full row)
                            nc.tensor.matmul(
                                pa[0][:, :cw],
                                qwT[0:64, hp, qsl],
                                kT_all[0:64, hp, jsl],
                                start=True, stop=True,
                            )
                            # e=1: AC then +BD via identity matmul (PE-only),
                            # exp straight from PSUM per chunk -- keeps the
                            # vector queue out of this head's score path.
                            nc.tensor.matmul(
                                pa[1][:, :cw],
                                qwT[64:128, hp, qsl],
                                kT_all[64:128, hp, jsl],
                                start=True, stop=False,
                            )
                            nc.tensor.matmul(
                                pa[1][:, :cw],
                                identb[:, :],
                                bd_sb[:, 1, jsl],
                                start=False, stop=True,
                            )
                            nc.vector.scalar_tensor_tensor(
                                bd_sb[:, 0, jsl], pa[0][:, :cw], 1.0,
                                bd_sb[:, 0, jsl], ALU.mult, ALU.add,
                            )
                            dnc = sm.tile([128, 1], f32, tag="dnc", name="dnc", bufs=6)
                            nc.scalar.activation(
                                prob[:, 1, jsl], pa[1][:, :cw], AF.Exp,
                                bias=0.0, scale=1.0, accum_out=dnc[:, :],
                            )
                            dn1.append(dnc)
                            cpos += cw

                        # e=1 softmax denom = sum of chunk denoms
                        for i in range(1, len(dn1)):
                            nc.vector.tensor_tensor(
                                dn1[0][:], dn1[0][:], dn1[i][:], ALU.add
                            )
                        rden1 = sm.tile([128, 1], f32, tag="rden", name="rden1")
                        nc.vector.reciprocal(rden1[:], dn1[0][:])
                        nc.vector.tensor_scalar(
                            out=prob[:, 1, :], in0=prob[:, 1, :],
                            scalar1=rden1[:], scalar2=None, op0=ALU.mult,
                        )
                        nc.sync.dma_start_transpose(probT[:, 1], prob[:, 1])
                        # e=0 softmax (full-row exp on ACT)
                        denom = sm.tile([128, 1], f32, tag="denom", name="denom")
                        nc.scalar.activation(
                            prob[:, 0, :wj], bd_sb[:, 0, :wj], AF.Exp,
                            bias=0.0, scale=1.0, accum_out=denom[:, :],
                        )
                        rden = sm.tile([128, 1], f32, tag="rden", name="rden")
                        nc.vector.reciprocal(rden[:], denom[:])
                        nc.vector.tensor_scalar(
                            out=prob[:, 0, :], in0=prob[:, 0, :],
                            scalar1=rden[:], scalar2=None, op0=ALU.mult,
                        )
                        nc.sync.dma_start_transpose(probT[:, 0], prob[:, 0])
                        # PV col-tiled into one PSUM bank, SEQUENTIAL per-head
                        # accumulation groups: e=1's start=True clears the
                        # bank's has_written bits but not e=0's finished data
                        # (nothing rewrites those partitions), so each group
                        # accumulates correctly while the groups' boundary
                        # matmuls overlap across col groups.
                        for e in (0, 1):
                            for t in range(njt):
                                nc.tensor.matmul(
                                    ppv[e * 64 : e * 64 + 64, qc0 : qc0 + 128],
                                    v_all[:, t, (h0 + e) * 64 : (h0 + e) * 64 + 64],
                                    probT[:, e, t, :],
                                    start=(t == 0),
                                    stop=(t == njt - 1),
                                    skip_group_check=True,
                                )
                    # previous pair's attnT copy, issued here so its (long
                    # satisfied) PV dependency never stalls a hot queue
                    flush_attnT()
                    ppv_prev[0] = (hp, ppv)

                for h0 in range(0, N_HEAD, 2):
                    e_pass_pair(h0)

                # ---- gathered K/V into SBUF, consolidated across ranks so
                # only a handful of DMA issues sit behind the AllGather.
                # rank r owns big rows [1952-128r, ...+128) and small rows
                # [32+128r, ...+128): 128-aligned j-blocks within the sliced
                # windows [1056, 2080) / [32, 1056). ----
                prev = issue_reads_pair(0)
                kT_all = kvp.tile([128, HP, T], bf16, tag="kTall")
                v_all = kvp.tile([128, 17, D_MODEL], bf16, tag="vall")
                nc.vector.memset(v_all[:, 16, :], 0.0)
                agk = ag_out[:, 0:kv_off].rearrange(
                    "r (hp p j) -> r hp p j", hp=HP, p=128, j=NK_TOK
                )
                agv = ag_out[:, kv_off:].rearrange(
                    "r (p c) -> r p c", c=D_MODEL
                )
                # kT micro+small first, then big, hp-ascending: pass2's AC
                # streams in j-order per hp, so subtile deps let pair 0 start
                # a few hundred KB after the AllGather instead of 4.7MB after.
                # (DMA APs allow at most 3 dims, so hp stays a python loop)
                nc.sync.dma_start(
                    kT_all[:, :, 0:32],
                    agk[0, :, :, 256:288].rearrange("hp p j -> p hp j"),
                )
                for hp in range(HP):
                    nc.sync.dma_start(
                        kT_all[:, hp, 32:1056].rearrange(
                            "p (rr j) -> p rr j", j=128
                        ),
                        agk[:, hp, :, 128:256].rearrange("r p j -> p r j"),
                    )
                    nc.sync.dma_start(
                        kT_all[:, hp, 1056:2080].rearrange(
                            "p (rr j) -> p rr j", j=128
                        ),
                        agk[::-1, hp, :, 0:128].rearrange("r p j -> p r j"),
                    )
                # v rows split 96/32 across adjacent 128-token tiles: 5 DMAs
                nc.sync.dma_start(
                    v_all[32:128, 8:16, :],
                    agv[::-1, 0:96, :].rearrange("r p c -> p r c"),
                )
                nc.sync.dma_start(
                    v_all[0:32, 9:17, :],
                    agv[::-1, 96:128, :].rearrange("r p c -> p r c"),
                )
                nc.sync.dma_start(
                    v_all[32:128, 0:8, :],
                    agv[:, 128:224, :].rearrange("r p c -> p r c"),
                )
                nc.sync.dma_start(
                    v_all[0:32, 1:9, :],
                    agv[:, 224:256, :].rearrange("r p c -> p r c"),
                )
                nc.sync.dma_start(v_all[0:32, 0, :], agv[0, 256:288, :])

                for h0 in range(0, N_HEAD, 2):
                    nxt = issue_reads_pair(h0 + 2) if h0 + 2 < N_HEAD else None
                    pass2_pair(h0, prev, first=(l == 0 and h0 == 0))
                    prev = nxt
                flush_attnT()

                # ---- Wo + residual + LN1 ----
                for qt in range(3):
                    pw = ps.tile([128, 512], f32, tag="pp")
                    for d in range(HP):
                        nc.tensor.matmul(
                            pw[:],
                            attnT[:, d, qt * 128 : (qt + 1) * 128],
                            wo_sb[:, d, :],
                            start=(d == 0), stop=(d == HP - 1),
                        )
                    x = sm.tile([128, D_MODEL], f32, tag="xres")
                    nc.vector.tensor_tensor(x[:], w_sb[:, qt, :], pw[:], ALU.add)
                    _layernorm(
                        nc, sm, w_sb[:, qt, :], x,
                        None if trivial_gb else gb_sb[:, 0, :],
                        None if trivial_gb else gb_sb[:, 1, :],
                        eps_sb[:],
                    )

                # ---- FFN ----
                w1T = wts.tile([128, HP, 384], bf16, tag="wT")
                for qt in range(3):
                    for d in range(HP):
                        pt = ps.tile([128, 512], f32, tag="pp")
                        nc.tensor.transpose(
                            pt[:, :128], w_sb[:, qt, d * 128 : (d + 1) * 128], ident[:]
                        )
                        nc.scalar.copy(w1T[:, d, qt * 128 : (qt + 1) * 128], pt[:, :128])
                pf = [
                    psff.tile([128, 512], f32, tag=f"pf{qt}", name=f"pf{qt}")
                    for qt in range(3)
                ]
                for di in range(16):
                    phh = ps.tile([128, 512], f32, tag="pp")
                    for d in range(HP):
                        nc.tensor.matmul(
                            phh[:, :384],
                            w1_sb[:, d, di * 128 : (di + 1) * 128],
                            w1T[:, d, :],
                            start=(d == 0), stop=(d == HP - 1),
                        )
                    h1t = mid.tile([128, 384], bf16, tag="h1t")
                    if trivial_b:
                        nc.scalar.activation(
                            h1t[:], phh[:, :384], AF.Relu, bias=0.0, scale=1.0
                        )
                    else:
                        nc.scalar.activation(
                            h1t[:], phh[:, :384], AF.Relu,
                            bias=b1_sb[:, di : di + 1], scale=1.0,
                        )
                    for qt in range(3):
                        nc.tensor.matmul(
                            pf[qt][:],
                            h1t[:, qt * 128 : (qt + 1) * 128],
                            w2_sb[:, di, :],
                            start=(di == 0), stop=(di == 15),
                        )
                for qt in range(3):
                    x = sm.tile([128, D_MODEL], f32, tag="xres")
                    if trivial_b:
                        nc.vector.tensor_tensor(x[:], pf[qt][:], w_sb[:, qt, :], ALU.add)
                    else:
                        nc.vector.scalar_tensor_tensor(
                            x[:], pf[qt][:], 1.0, b2_sb[:], ALU.mult, ALU.add
                        )
                        nc.vector.tensor_tensor(x[:], x[:], w_sb[:, qt, :], ALU.add)
                    _layernorm(
                        nc, sm, w_sb[:, qt, :], x,
                        None if trivial_gb else gb_sb[:, 2, :],
                        None if trivial_gb else gb_sb[:, 3, :],
                        eps_sb[:],
                    )

            for qt in range(3):
                nc.sync.dma_start(
                    out_t[qt * 128 : (qt + 1) * 128, :], w_sb[:, qt, :]
                )

    nc.compile()
    return nc


_NC_CACHE = {}
LAST_RESULT = None


def kernel(**inputs):
    global LAST_RESULT
    trivial_gb = (
        np.all(np.asarray(inputs["ln1_scale"]) == 1.0)
        and np.all(np.asarray(inputs["ln2_scale"]) == 1.0)
        and np.all(np.asarray(inputs["ln1_bias"]) == 0.0)
        and np.all(np.asarray(inputs["ln2_bias"]) == 0.0)
    )
    trivial_b = (
        np.all(np.asarray(inputs["ffn_b1"]) == 0.0)
        and np.all(np.asarray(inputs["ffn_b2"]) == 0.0)
    )
    per_core = _host_prep(inputs)
    drop = []
    if trivial_gb:
        drop.append("gb")
    if trivial_b:
        drop += ["b1col", "b2bc"]
    for pc in per_core:
        for k in drop:
            pc.pop(k, None)
    key = (trivial_gb, trivial_b)
    if key not in _NC_CACHE:
        _NC_CACHE[key] = _build(trivial_gb=trivial_gb, trivial_b=trivial_b)
    res = run_bass_kernel_spmd(
        _NC_CACHE[key], [dict(pc) for pc in per_core], core_ids=list(range(N_CORES)),
        tmpdir=os.environ.get("BASS_TMPDIR") or None,
    )
    LAST_RESULT = res
    out = np.zeros((T, D_MODEL), np.float32)
    for m in range(N_CORES):
        rb, rs = _row_bases(m)
        wout = res.results[m]["wout"]
        out[rb : rb + 128] = wout[0:128]
        out[rs : rs + 128] = wout[128:256]
        if m == 0:
            out[0:32] = wout[256:288]
    return np.ascontiguousarray(out[:, None, :].astype(np.float32))



# revision 43
# speedup vs baseline: 1.0314x; 1.0314x over previous
"""Trainium2 Bass kernel for nn_MemTransformerLM (Transformer-XL rel-attention).

v3 on top of v2: rT host-folded (pos_emb @ Wr is weight-only), k/v projected
before q so the AllGather hides behind q-proj + the E pass, weight/KV-gather
DMAs moved to the SWDGE (gpsimd) queue so the sync queue never stalls on the
collective, E matmuls issued in head pairs (row-group concurrency), minimal
P-buffer init.

v2: causally-truncated, load-balanced attention. Every core runs the same
program (SPMD single NEFF) over three 128-row "slots" whose row content is
core-dependent:
  - big   slot: rows [1952-128*pid, 2080-128*pid), scores over j in [0, 2080)
  - small slot: rows [  32+128*pid,  160+128*pid), scores over j in [0, 1056)
  - micro slot: rows [0, 32) (replicated on all cores; core 0's output used),
                scores over j in [0, 128)
Per-core score columns: 2080+1056+128 = 3264 vs 3*2080 = 6240 for the naive
contiguous-span split -- ~1.9x less attention work, perfectly balanced.

The Transformer-XL _rel_shift uses the padded-DRAM-pitch trick: per (head,
slot) a private DRAM buffer [129, 2081]; E rows are written at row pitch
T+1 = 2081 behind a zero column and read back at row stride T with a
rank-dependent offset, reproducing the reference's wrap semantics (including
the "garbage" upper-triangle values inside the two mem-token corners).
Private per-(head,slot) buffers keep the write->read round trips of
different heads independent so the pipeline overlaps across heads.

Masking: big/micro slots use host-precomputed additive masks. The small slot
needs no mask at all: its masked positions (j > i) always wrap into the
never-written low-column region of its P buffer, which is poisoned once with
-1e30 at kernel start.
"""

import os

import numpy as np
import ml_dtypes

import concourse.bass as bass
import concourse.mybir as mybir
import concourse.tile as tile
from concourse import bacc
from concourse.bass import ds
from concourse.bass_utils import run_bass_kernel_spmd
from concourse.masks import make_identity

BF16 = ml_dtypes.bfloat16
DT = mybir.dt
AF = mybir.ActivationFunctionType
ALU = mybir.AluOpType

N_LAYER = 4
N_HEAD = 8
D_HEAD = 64
D_MODEL = 512
D_INNER = 2048
NMT = 16
T = 2048 + 2 * NMT      # 2080
N_CORES = 8
NEG = -1e30
SCALE = 1.0 / np.sqrt(D_HEAD)
HP = 4                  # 128-partition tiles over D_MODEL

# slots: (q-col base, score width, padded prob width, n j-tiles, E k-window lo)
W_BIG = 2080
W_SMALL = 1056
W_MICRO = 128
PAD_BIG = 17 * 128      # 2176
PAD_SMALL = 9 * 128     # 1152
CH_BIG = [512, 512, 512, 512, 32]
CH_SMALL = [512, 512, 32]
PROWS = 129             # 128 rows + 1 wrap-margin row per (head, slot) buffer
PITCH = T + 1           # 2081
POISON_W = 1025         # small-slot poison columns [0, POISON_W)
MASKB_W = 1152          # big-slot mask covers j in [W_BIG-MASKB_W, W_BIG) only

NK_TOK = 288            # tokens contributed per core to the allgather (128+128+32)
AG_N = D_MODEL * NK_TOK + NK_TOK * D_MODEL  # kT part + v part, elements


def _row_bases(m):
    return 1952 - 128 * m, 32 + 128 * m  # big, small global row starts


def _host_prep(inputs):
    word_emb = np.asarray(inputs["word_emb"], np.float32)
    mem_tokens = np.asarray(inputs["mem_tokens"], np.float32)
    w = np.transpose(word_emb, (1, 0, 2))[:, 0, :]
    mem = mem_tokens[:, 0, :]
    w_full = np.concatenate([mem, w, mem], axis=0)          # [T, 512]

    inv_freq = 1.0 / (10000.0 ** (np.arange(0, D_MODEL, 2, dtype=np.float32) / D_MODEL))
    pos_seq = np.arange(T - 1, -1, -1.0, dtype=np.float32)
    sinusoid = pos_seq[:, None] * inv_freq[None, :]
    pos_emb = np.concatenate([np.sin(sinusoid), np.cos(sinusoid)], axis=-1)

    wqkv = np.asarray(inputs["Wqkv"], np.float32).astype(BF16)
    # r projection depends only on weights + the fixed sinusoid table:
    # fold it on the host, ship rT = (pos_emb @ Wr[l]).T per layer.
    wr_f32 = np.asarray(inputs["Wr"], np.float32)
    rT = np.ascontiguousarray(
        np.stack([(pos_emb @ wr_f32[l]).T for l in range(N_LAYER)])
    ).astype(BF16)                                          # [L, 512, T]
    wo = np.asarray(inputs["Wo"], np.float32).astype(BF16)
    w1 = np.asarray(inputs["ffn_W1"], np.float32).astype(BF16)
    w2 = np.asarray(inputs["ffn_W2"], np.float32).astype(BF16)
    # fold the 1/sqrt(d) score scale into q (and its biases)
    rwb = np.ascontiguousarray(
        (np.asarray(inputs["r_w_bias"], np.float32) * SCALE).reshape(-1, 1))
    rrb = np.ascontiguousarray(
        (np.asarray(inputs["r_r_bias"], np.float32) * SCALE).reshape(-1, 1))

    # reference mask, True = masked
    M = np.triu(np.ones((T, T), dtype=bool), k=1)
    M[:NMT, :NMT] = False
    M[-NMT:, -NMT:] = False

    mask_micro = np.zeros((128, W_MICRO), np.float32)
    mask_micro[:32, :] = np.where(M[:32, :W_MICRO], NEG, 0.0)

    per_core = []
    for rank in range(N_CORES):
        rb, rs = _row_bases(rank)
        w0 = np.zeros((384, D_MODEL), np.float32)
        w0[0:128] = w_full[rb : rb + 128]
        w0[128:256] = w_full[rs : rs + 128]
        w0[256:288] = w_full[0:32]
        mask_big = np.where(M[rb : rb + 128, W_BIG - MASKB_W :], NEG, 0.0).astype(BF16)
        assert not M[rb : rb + 128, : W_BIG - MASKB_W].any()
        per_core.append(
            {
                "w0": w0,
                "rT": rT,
                "wqkv": wqkv,
                "wo": wo,
                "w1": w1,
                "w2": w2,
                "rwb": rwb,
                "rrb": rrb,
                "mask_big": np.ascontiguousarray(mask_big),
                "mask_micro": np.ascontiguousarray(mask_micro.astype(BF16)),
            }
        )

    gb = np.ascontiguousarray(
        np.stack(
            [
                np.broadcast_to(np.asarray(inputs["ln1_scale"], np.float32)[:, None, :],
                                (N_LAYER, 128, D_MODEL)),
                np.broadcast_to(np.asarray(inputs["ln1_bias"], np.float32)[:, None, :],
                                (N_LAYER, 128, D_MODEL)),
                np.broadcast_to(np.asarray(inputs["ln2_scale"], np.float32)[:, None, :],
                                (N_LAYER, 128, D_MODEL)),
                np.broadcast_to(np.asarray(inputs["ln2_bias"], np.float32)[:, None, :],
                                (N_LAYER, 128, D_MODEL)),
            ],
            axis=2,
        ).astype(np.float32)
    )
    b1col = np.ascontiguousarray(
        np.asarray(inputs["ffn_b1"], np.float32).reshape(N_LAYER, D_INNER, 1))
    b2bc = np.ascontiguousarray(
        np.broadcast_to(np.asarray(inputs["ffn_b2"], np.float32)[:, None, :],
                        (N_LAYER, 128, D_MODEL)).copy())
    for pc in per_core:
        pc["gb"] = gb
        pc["b1col"] = b1col
        pc["b2bc"] = b2bc
    return per_core


def _layernorm(nc, sm, out_ap, x, g, b, eps):
    f32 = DT.float32
    stats = sm.tile([128, 6], f32, tag="lnst")
    mv = sm.tile([128, 2], f32, tag="lnmv")
    nc.vector.bn_stats(stats[:], x[:])
    nc.vector.bn_aggr(mv[:], stats[:])
    std = sm.tile([128, 1], f32, tag="lnstd")
    nc.scalar.activation(std[:], mv[:, 1:2], AF.Sqrt, bias=eps, scale=1.0)
    rstd = sm.tile([128, 1], f32, tag="lnrstd")
    nc.vector.reciprocal(rstd[:], std[:])
    if g is None:
        nc.vector.tensor_scalar(
            out=out_ap, in0=x[:], scalar1=mv[:, 0:1], scalar2=rstd[:],
            op0=ALU.subtract, op1=ALU.mult,
        )
    else:
        xn = sm.tile([128, D_MODEL], f32, tag="lnxn")
        nc.vector.tensor_scalar(
            out=xn[:], in0=x[:], scalar1=mv[:, 0:1], scalar2=rstd[:],
            op0=ALU.subtract, op1=ALU.mult,
        )
        nc.vector.tensor_tensor(xn[:], xn[:], g, ALU.mult)
        nc.vector.tensor_tensor(out_ap, xn[:], b, ALU.add)


def _build(trivial_gb=True, trivial_b=True):
    nc = bacc.Bacc("TRN2", num_devices=N_CORES, dynamic_dma_scratch_size=4096)
    f32, bf16 = DT.float32, DT.bfloat16

    w0_t = nc.dram_tensor("w0", [384, D_MODEL], f32, kind="ExternalInput")
    rT_t = nc.dram_tensor("rT", [N_LAYER, D_MODEL, T], bf16, kind="ExternalInput")
    wqkv_t = nc.dram_tensor("wqkv", [N_LAYER, D_MODEL, 3 * D_MODEL], bf16, kind="ExternalInput")
    wo_t = nc.dram_tensor("wo", [N_LAYER, D_MODEL, D_MODEL], bf16, kind="ExternalInput")
    w1_t = nc.dram_tensor("w1", [N_LAYER, D_MODEL, D_INNER], bf16, kind="ExternalInput")
    w2_t = nc.dram_tensor("w2", [N_LAYER, D_INNER, D_MODEL], bf16, kind="ExternalInput")
    rwb_t = nc.dram_tensor("rwb", [D_MODEL, 1], f32, kind="ExternalInput")
    rrb_t = nc.dram_tensor("rrb", [D_MODEL, 1], f32, kind="ExternalInput")
    maskb_t = nc.dram_tensor("mask_big", [128, MASKB_W], bf16, kind="ExternalInput")
    maskm_t = nc.dram_tensor("mask_micro", [128, W_MICRO], bf16, kind="ExternalInput")
    gb_t = None if trivial_gb else nc.dram_tensor(
        "gb", [N_LAYER, 128, 4, D_MODEL], f32, kind="ExternalInput")
    b1_t = b2_t = None
    if not trivial_b:
        b1_t = nc.dram_tensor("b1col", [N_LAYER, D_INNER, 1], f32, kind="ExternalInput")
        b2_t = nc.dram_tensor("b2bc", [N_LAYER, 128, D_MODEL], f32, kind="ExternalInput")
    out_t = nc.dram_tensor("wout", [384, D_MODEL], f32, kind="ExternalOutput")

    # per-(head-pair, slot) rel-shift buffers, [2, PROWS, PITCH] flat so one
    # DMA serves both heads of a pair
    PP = PROWS * PITCH
    p_big = [nc.dram_tensor(f"pb{q}", [2 * PP], bf16, kind="Internal")
             for q in range(N_HEAD // 2)]
    p_small = [nc.dram_tensor(f"psm{q}", [2 * PP], bf16, kind="Internal")
               for q in range(N_HEAD // 2)]
    p_micro = [nc.dram_tensor(f"pmi{q}", [2 * PP], bf16, kind="Internal")
               for q in range(N_HEAD // 2)]

    ag_in = nc.dram_tensor("ag_in", [AG_N], bf16, kind="Internal")
    ag_out = nc.dram_tensor("ag_out", [N_CORES, AG_N], bf16, kind="Internal",
                            addr_space="Shared")
    rg = [list(range(N_CORES))]

    kv_off = D_MODEL * NK_TOK
    agin_k = ag_in[0:kv_off].rearrange("(a b) -> a b", b=NK_TOK)      # [512, 288]
    agin_v = ag_in[kv_off:].rearrange("(a b) -> a b", b=D_MODEL)      # [288, 512]

    with tile.TileContext(nc, num_cores=N_CORES) as tc:
        pid = nc.sync.partition_id()
        with (
            tc.tile_pool(name="const", bufs=1) as constp,
            tc.tile_pool(name="pers", bufs=1) as pers,
            tc.tile_pool(name="wts", bufs=1) as wts,
            tc.tile_pool(name="kv", bufs=1) as kvp,
            tc.tile_pool(name="mid", bufs=2) as mid,
            tc.tile_pool(name="epool", bufs=2) as epool,
            tc.tile_pool(name="bdp", bufs=2) as bdp,
            tc.tile_pool(name="probp", bufs=1) as probp,
            tc.tile_pool(name="ptp", bufs=1) as ptp,
            tc.tile_pool(name="sm", bufs=2) as sm,
            tc.tile_pool(name="ps", bufs=3, space="PSUM") as ps,
            tc.tile_pool(name="pspv", bufs=2, space="PSUM") as pspv,
            tc.tile_pool(name="psff", bufs=1, space="PSUM") as psff,
        ):
            ident = constp.tile([128, 128], f32)
            make_identity(nc, ident[:])

            # ---- init P buffers (minimal read-coverage init, verified by
            # symbolic read-region check; see sim_check.py) ----
            zrow = epool.tile([128, W_BIG], bf16, tag="esb")
            nc.vector.memset(zrow[:], 0.0)
            for q in range(N_HEAD // 2):
                for e in (0, 1):
                    # big: zero-pad col 0 (rows 0-128) + wrap row 128
                    v2 = p_big[q][e * PP : (e + 1) * PP].rearrange(
                        "(r c) -> r c", c=PITCH)
                    nc.sync.dma_start(v2[0:128, 0:1], zrow[:, 0:1])
                    nc.sync.dma_start(v2[128:129, 0:W_BIG], zrow[:1, :])
                    nc.sync.dma_start(v2[128:129, W_BIG:PITCH], zrow[:1, 0:1])
                    # micro: rows 0-33 wide + rows 34-128 low cols
                    v2 = p_micro[q][e * PP : (e + 1) * PP].rearrange(
                        "(r c) -> r c", c=PITCH)
                    nc.sync.dma_start(v2[0:34, 0:1953], zrow[:34, :1953])
                    nc.sync.dma_start(v2[34:129, 0:128], zrow[:95, :128])
            prow = epool.tile([128, W_BIG], bf16, tag="esb")
            nc.vector.memset(prow[:], NEG)
            for q in range(N_HEAD // 2):
                for e in (0, 1):
                    v2 = p_small[q][e * PP : (e + 1) * PP].rearrange(
                        "(r c) -> r c", c=PITCH)
                    nc.sync.dma_start(v2[0:128, 0:POISON_W], prow[:, :POISON_W])
                    nc.sync.dma_start(v2[128:129, 0:POISON_W], prow[:1, :POISON_W])

            rwb_sb = pers.tile([128, HP], f32)
            rrb_sb = pers.tile([128, HP], f32)
            for d in range(HP):
                nc.sync.dma_start(rwb_sb[:, d : d + 1], rwb_t[d * 128 : (d + 1) * 128, :])
                nc.sync.dma_start(rrb_sb[:, d : d + 1], rrb_t[d * 128 : (d + 1) * 128, :])

            eps_sb = pers.tile([128, 1], f32)
            nc.vector.memset(eps_sb[:], 1e-5)
            maskb_sb = pers.tile([128, MASKB_W], bf16)
            nc.sync.dma_start(maskb_sb[:], maskb_t[:])
            maskm_sb = pers.tile([128, W_MICRO], bf16)
            nc.sync.dma_start(maskm_sb[:], maskm_t[:])
            w_sb = pers.tile([128, 3, D_MODEL], f32)
            for qt in range(3):
                nc.sync.dma_start(w_sb[:, qt, :], w0_t[qt * 128 : (qt + 1) * 128, :])

            for l in range(N_LAYER):
                # ---- layer weights + host-folded rT (SWDGE queue, off the
                # sync engine so they overlap the attention DMA pipeline) ----
                wqkv_sb = wts.tile([128, HP, 3 * D_MODEL], bf16, tag="wqkv")
                wo_sb = wts.tile([128, HP, D_MODEL], bf16, tag="wol")
                w1_sb = wts.tile([128, HP, D_INNER], bf16, tag="w1l")
                w2_sb = wts.tile([128, 16, D_MODEL], bf16, tag="w2l")
                rT_sb = wts.tile([128, HP, T], bf16, tag="rT")
                # one consolidated DMA per weight tensor: src dims (p, d, c)
                nc.sync.dma_start(
                    wqkv_sb[:, :, :],
                    wqkv_t[l].rearrange("(d p) c -> p d c", p=128),
                )
                nc.sync.dma_start(
                    rT_sb[:, :, :], rT_t[l].rearrange("(d p) c -> p d c", p=128)
                )
                nc.sync.dma_start(
                    wo_sb[:, :, :], wo_t[l].rearrange("(d p) c -> p d c", p=128)
                )
                nc.sync.dma_start(
                    w1_sb[:, :, :], w1_t[l].rearrange("(d p) c -> p d c", p=128)
                )
                nc.sync.dma_start(
                    w2_sb[:, :, :], w2_t[l].rearrange("(d p) c -> p d c", p=128)
                )
                gb_sb = None
                if not trivial_gb:
                    gb_sb = wts.tile([128, 4, D_MODEL], f32, tag="gbl")
                    nc.sync.dma_start(gb_sb[:], gb_t[l])
                b1_sb = b2_sb = None
                if not trivial_b:
                    b1_sb = wts.tile([128, 16], f32, tag="b1l")
                    for d in range(16):
                        nc.sync.dma_start(b1_sb[:, d : d + 1], b1_t[l, d * 128 : (d + 1) * 128, :])
                    b2_sb = wts.tile([128, D_MODEL], f32, tag="b2l")
                    nc.sync.dma_start(b2_sb[:], b2_t[l])

                # ---- transpose residual -> wT bf16 ----
                wT_sb = wts.tile([128, HP, 384], bf16, tag="wT")
                for qt in range(3):
                    for d in range(HP):
                        pt = ps.tile([128, 512], f32, tag="pp")
                        nc.tensor.transpose(
                            pt[:, :128], w_sb[:, qt, d * 128 : (d + 1) * 128], ident[:]
                        )
                        nc.scalar.copy(wT_sb[:, d, qt * 128 : (qt + 1) * 128], pt[:, :128])

                # ---- k/v projections first so the AllGather can launch
                # early and hide behind q-proj + the whole E pass ----
                kT_own = wts.tile([128, HP, NK_TOK], bf16, tag="kTown")
                for hp in range(HP):
                    pk = ps.tile([128, 512], f32, tag="pp")
                    for d in range(HP):
                        nc.tensor.matmul(
                            pk[:, :384],
                            wqkv_sb[:, d, D_MODEL + hp * 128 : D_MODEL + hp * 128 + 128],
                            wT_sb[:, d, :],
                            start=(d == 0), stop=(d == HP - 1),
                        )
                    nc.scalar.copy(kT_own[:, hp, :], pk[:, :NK_TOK])
                nc.sync.dma_start(
                    agin_k.rearrange("(hp p) c -> p hp c", p=128), kT_own[:, :, :]
                )
                v_own = wts.tile([128, 3, D_MODEL], bf16, tag="vown")
                for qt in range(3):
                    pv = ps.tile([128, 512], f32, tag="pp")
                    for d in range(HP):
                        nc.tensor.matmul(
                            pv[:],
                            wT_sb[:, d, qt * 128 : (qt + 1) * 128],
                            wqkv_sb[:, d, 2 * D_MODEL :],
                            start=(d == 0), stop=(d == HP - 1),
                        )
                    nc.vector.tensor_copy(v_own[:, qt, :], pv[:])
                    rows = 32 if qt == 2 else 128
                    nc.sync.dma_start(
                        agin_v[qt * 128 : qt * 128 + rows, :], v_own[:rows, qt, :]
                    )

                nc.gpsimd.collective_compute(
                    "AllGather", ALU.bypass, replica_groups=rg,
                    ins=[ag_in[:]], outs=[ag_out[:]],
                )

                # ---- q projection (pre-scaled by 1/sqrt(d)) ----
                qwT = wts.tile([128, HP, 384], bf16, tag="qwT")
                qrT = wts.tile([128, HP, 384], bf16, tag="qrT")
                for hp in range(HP):
                    pq = ps.tile([128, 512], f32, tag="pp")
                    for d in range(HP):
                        nc.tensor.matmul(
                            pq[:, :384],
                            wqkv_sb[:, d, hp * 128 : hp * 128 + 128],
                            wT_sb[:, d, :],
                            start=(d == 0), stop=(d == HP - 1),
                        )
                    nc.scalar.activation(
                        qwT[:, hp, :], pq[:, :384], AF.Identity,
                        bias=rwb_sb[:, hp : hp + 1], scale=float(SCALE),
                    )
                    nc.scalar.activation(
                        qrT[:, hp, :], pq[:, :384], AF.Identity,
                        bias=rrb_sb[:, hp : hp + 1], scale=float(SCALE),
                    )

                # ---- attention ----
                # slot parameters: (qcol0, width, padded width, chunks,
                #                   k-window lo, p-tensor list, probT tile0, n tiles)
                def slot_params(h, si):
                    q = h // 2
                    if si == 0:
                        return (0, W_BIG, PAD_BIG, CH_BIG, 0, p_big[q],
                                128 + pid * 128, 0, 17)
                    if si == 1:
                        return (128, W_SMALL, PAD_SMALL, CH_SMALL, T - W_SMALL,
                                p_small[q], 2048 - pid * 128, 17, 9)
                    return (256, W_MICRO, W_MICRO, [128], T - W_MICRO,
                            p_micro[q], 2080, 26, 1)

                # E computation for a PAIR of heads -> DRAM. The pair's lhsT
                # base partitions are 0 / 64, so bass auto-derives row-group
                # tile positions and the two K=64 matmuls run concurrently in
                # the PE array. PSUM evacuation alternates scalar/vector.
                def e_pass_pair(h0):
                    hp = h0 // 2
                    for si in range(3):
                        qc0, wj, wpad, chs, klo, pt_t, off, jt0, njt = slot_params(h0, si)
                        # pair view [row, e, col] so one DMA writes both heads
                        p3d = pt_t.rearrange("(e r c) -> r e c", e=2, c=PITCH)
                        qsl = slice(qc0, qc0 + 128)
                        e_sb = epool.tile([128, 2, W_BIG], bf16, tag="esb", name="e_sb")
                        cpos = 0
                        for cw in chs:
                            pe = [
                                ps.tile([128, 512], f32, tag="pp", name="pe0"),
                                ps.tile([128, 512], f32, tag="pp", name="pe1"),
                            ]
                            for e in (0, 1):
                                nc.tensor.matmul(
                                    pe[e][:, :cw],
                                    qrT[e * 64 : e * 64 + 64, hp, qsl],
                                    rT_sb[e * 64 : e * 64 + 64, hp, klo + cpos : klo + cpos + cw],
                                    start=True, stop=True,
                                )
                            nc.scalar.copy(e_sb[:, 0, cpos : cpos + cw], pe[0][:, :cw])
                            nc.vector.tensor_copy(e_sb[:, 1, cpos : cpos + cw], pe[1][:, :cw])
                            cpos += cw
                        if si == 2:
                            # corner-garbage columns k in [0, 16)
                            pe = [
                                ps.tile([128, 512], f32, tag="pp", name="pec0"),
                                ps.tile([128, 512], f32, tag="pp", name="pec1"),
                            ]
                            for e in (0, 1):
                                nc.tensor.matmul(
                                    pe[e][:, :16],
                                    qrT[e * 64 : e * 64 + 64, hp, qsl],
                                    rT_sb[e * 64 : e * 64 + 64, hp, 0:16],
                                    start=True, stop=True,
                                )
                            ec = mid.tile([128, 2, 16], bf16, tag="ecrn")
                            nc.scalar.copy(ec[:, 0, :], pe[0][:, :16])
                            nc.vector.tensor_copy(ec[:, 1, :], pe[1][:, :16])
                            nc.sync.dma_start(p3d[0:128, :, 1:17], ec[:, :, :])
                        nc.sync.dma_start(
                            p3d[0:128, :, 1 + klo : 1 + klo + wj], e_sb[:, :, :wj]
                        )

                # pass 2: shifted read-back, scores, softmax, PV.
                # BD reads are prefetched one head ahead so they are never
                # queued behind the probT transposes on the sync queue.
                attnT = wts.tile([128, HP, 384], bf16, tag="attnT")
                BD_TAGS = ["bdb", "bds", "bdm"]
                BD_W = [W_BIG, W_SMALL, W_MICRO]

                def issue_reads_pair(h0):
                    bds = []
                    for si in range(3):
                        sp = slot_params(h0, si)
                        wj, off, pt_t = sp[1], sp[6], sp[5]
                        bd_sb = bdp.tile([128, 2, BD_W[si]], bf16, tag=BD_TAGS[si])
                        src_ap = pt_t.rearrange("(e x) -> e x", e=2)[
                            :, ds(off, 128 * T)
                        ].rearrange("e (p j) -> p e j", j=T)
                        nc.sync.dma_start(bd_sb[:, :, :wj], src_ap[:, :, :wj])
                        bds.append(bd_sb)
                    return bds

                ppv_prev = [None]

                def flush_attnT():
                    if ppv_prev[0] is not None:
                        php, ptile = ppv_prev[0]
                        nc.vector.tensor_copy(attnT[:, php, :], ptile[:])
                        ppv_prev[0] = None

                def pass2_pair(h0, bds, first):
                    hp = h0 // 2
                    ppv = pspv.tile([128, 384], f32, tag="ppv", name="ppv")
                    for si in range(3):
                        qc0, wj, wpad, chs, klo, _, off, jt0, njt = slot_params(h0, si)
                        qsl = slice(qc0, qc0 + 128)
                        bd_sb = bds[si]

                        # scores: AC + BD (+ mask for big/micro; small is
                        # handled entirely by the poison region)
                        for e in (0, 1):
                            if si == 0:
                                nc.vector.tensor_tensor(
                                    bd_sb[:, e, W_BIG - MASKB_W :],
                                    bd_sb[:, e, W_BIG - MASKB_W :], maskb_sb[:], ALU.add,
                                )
                            elif si == 2:
                                nc.vector.tensor_tensor(
                                    bd_sb[:, e, :wj], bd_sb[:, e, :wj],
                                    maskm_sb[:, :wj], ALU.add,
                                )
                        cpos = 0
                        for cw in chs:
                            jsl = slice(cpos, cpos + cw)
                            pa = [
                                ps.tile([128, 512], f32, tag="pp", name="pa0"),
                                ps.tile([128, 512], f32, tag="pp", name="pa1"),
                            ]
                            for e in (0, 1):
                                nc.tensor.matmul(
                                    pa[e][:, :cw],
                                    qwT[e * 64 : e * 64 + 64, hp, qsl],
                                    kT_all[e * 64 : e * 64 + 64, hp, jsl],
                                    start=True, stop=True,
                                )
                            for e in (0, 1):
                                nc.vector.scalar_tensor_tensor(
                                    bd_sb[:, e, jsl], pa[e][:, :cw], 1.0,
                                    bd_sb[:, e, jsl], ALU.mult, ALU.add,
                                )
                            cpos += cw

                        # softmax over computed j range
                        prob = probp.tile([128, 2, wpad], bf16, tag=f"prob{si}")
                        probT = ptp.tile([128, 2, njt, 128], bf16, tag=f"probT{si}")
                        for e in (0, 1):
                            denom = sm.tile([128, 1], f32, tag="denom", name="denom")
                            if wpad > wj and first:
                                # pad cols stay zero across heads/layers
                                # (x*rden keeps 0 at 0)
                                nc.vector.memset(prob[:, e, wj:wpad], 0.0)
                            nc.scalar.activation(
                                prob[:, e, :wj], bd_sb[:, e, :wj], AF.Exp,
                                bias=0.0, scale=1.0, accum_out=denom[:, :],
                            )
                            rden = sm.tile([128, 1], f32, tag="rden", name="rden")
                            nc.vector.reciprocal(rden[:], denom[:])
                            nc.vector.tensor_scalar(
                                out=prob[:, e, :], in0=prob[:, e, :],
                                scalar1=rden[:], scalar2=None, op0=ALU.mult,
                            )
                        # one xbar transpose covers both heads: the pair's
                        # concatenated [128, 2*wpad] source transposes into
                        # exactly the [128, 2, njt, 128] tile layout PV reads
                        nc.sync.dma_start_transpose(probT[:, :], prob[:, :])
                        # PV col-tiled into one PSUM bank, SEQUENTIAL per-head
                        # accumulation groups: e=1's start=True clears the
                        # bank's has_written bits but not e=0's finished data
                        # (nothing rewrites those partitions), so each group
                        # accumulates correctly while the groups' boundary
                        # matmuls overlap across col groups.
                        for e in (0, 1):
                            for t in range(njt):
                                nc.tensor.matmul(
                                    ppv[e * 64 : e * 64 + 64, qc0 : qc0 + 128],
                                    v_all[:, t, (h0 + e) * 64 : (h0 + e) * 64 + 64],
                                    probT[:, e, t, :],
                                    start=(t == 0),
                                    stop=(t == njt - 1),
                                    skip_group_check=True,
                                )
                    # previous pair's attnT copy, issued here so its (long
                    # satisfied) PV dependency never stalls a hot queue
                    flush_attnT()
                    ppv_prev[0] = (hp, ppv)

                for h0 in range(0, N_HEAD, 2):
                    e_pass_pair(h0)

                # ---- gathered K/V into SBUF, consolidated across ranks so
                # only a handful of DMA issues sit behind the AllGather.
                # rank r owns big rows [1952-128r, ...+128) and small rows
                # [32+128r, ...+128): 128-aligned j-blocks within the sliced
                # windows [1056, 2080) / [32, 1056). ----
                prev = issue_reads_pair(0)
                kT_all = kvp.tile([128, HP, T], bf16, tag="kTall")
                v_all = kvp.tile([128, 17, D_MODEL], bf16, tag="vall")
                nc.vector.memset(v_all[:, 16, :], 0.0)
                agk = ag_out[:, 0:kv_off].rearrange(
                    "r (hp p j) -> r hp p j", hp=HP, p=128, j=NK_TOK
                )
                agv = ag_out[:, kv_off:].rearrange(
                    "r (p c) -> r p c", c=D_MODEL
                )
                # kT micro+small first, then big, hp-ascending: pass2's AC
                # streams in j-order per hp, so subtile deps let pair 0 start
                # a few hundred KB after the AllGather instead of 4.7MB after.
                # (DMA APs allow at most 3 dims, so hp stays a python loop)
                nc.sync.dma_start(
                    kT_all[:, :, 0:32],
                    agk[0, :, :, 256:288].rearrange("hp p j -> p hp j"),
                )
                for hp in range(HP):
                    nc.sync.dma_start(
                        kT_all[:, hp, 32:1056].rearrange(
                            "p (rr j) -> p rr j", j=128
                        ),
                        agk[:, hp, :, 128:256].rearrange("r p j -> p r j"),
                    )
                    nc.sync.dma_start(
                        kT_all[:, hp, 1056:2080].rearrange(
                            "p (rr j) -> p rr j", j=128
                        ),
                        agk[::-1, hp, :, 0:128].rearrange("r p j -> p r j"),
                    )
                # v rows split 96/32 across adjacent 128-token tiles: 5 DMAs
                nc.sync.dma_start(
                    v_all[32:128, 8:16, :],
                    agv[::-1, 0:96, :].rearrange("r p c -> p r c"),
                )
                nc.sync.dma_start(
                    v_all[0:32, 9:17, :],
                    agv[::-1, 96:128, :].rearrange("r p c -> p r c"),
                )
                nc.sync.dma_start(
                    v_all[32:128, 0:8, :],
                    agv[:, 128:224, :].rearrange("r p c -> p r c"),
                )
                nc.sync.dma_start(
                    v_all[0:32, 1:9, :],
                    agv[:, 224:256, :].rearrange("r p c -> p r c"),
                )
                nc.sync.dma_start(v_all[0:32, 0, :], agv[0, 256:288, :])

                for h0 in range(0, N_HEAD, 2):
                    nxt = issue_reads_pair(h0 + 2) if h0 + 2 < N_HEAD else None
                    pass2_pair(h0, prev, first=(l == 0 and h0 == 0))
                    prev = nxt
                flush_attnT()

                # ---- Wo + residual + LN1 ----
                for qt in range(3):
                    pw = ps.tile([128, 512], f32, tag="pp")
                    for d in range(HP):
                        nc.tensor.matmul(
                            pw[:],
                            attnT[:, d, qt * 128 : (qt + 1) * 128],
                            wo_sb[:, d, :],
                            start=(d == 0), stop=(d == HP - 1),
                        )
                    x = sm.tile([128, D_MODEL], f32, tag="xres")
                    nc.vector.tensor_tensor(x[:], w_sb[:, qt, :], pw[:], ALU.add)
                    _layernorm(
                        nc, sm, w_sb[:, qt, :], x,
                        None if trivial_gb else gb_sb[:, 0, :],
                        None if trivial_gb else gb_sb[:, 1, :],
                        eps_sb[:],
                    )

                # ---- FFN ----
                w1T = wts.tile([128, HP, 384], bf16, tag="wT")
                for qt in range(3):
                    for d in range(HP):
                        pt = ps.tile([128, 512], f32, tag="pp")
                        nc.tensor.transpose(
                            pt[:, :128], w_sb[:, qt, d * 128 : (d + 1) * 128], ident[:]
                        )
                        nc.scalar.copy(w1T[:, d, qt * 128 : (qt + 1) * 128], pt[:, :128])
                pf = [
                    psff.tile([128, 512], f32, tag=f"pf{qt}", name=f"pf{qt}")
                    for qt in range(3)
                ]
                for di in range(16):
                    phh = ps.tile([128, 512], f32, tag="pp")
                    for d in range(HP):
                        nc.tensor.matmul(
                            phh[:, :384],
                            w1_sb[:, d, di * 128 : (di + 1) * 128],
                            w1T[:, d, :],
                            start=(d == 0), stop=(d == HP - 1),
                        )
                    h1t = mid.tile([128, 384], bf16, tag="h1t")
                    if trivial_b:
                        nc.scalar.activation(
                            h1t[:], phh[:, :384], AF.Relu, bias=0.0, scale=1.0
                        )
                    else:
                        nc.scalar.activation(
                            h1t[:], phh[:, :384], AF.Relu,
                            bias=b1_sb[:, di : di + 1], scale=1.0,
                        )
                    for qt in range(3):
                        nc.tensor.matmul(
                            pf[qt][:],
                            h1t[:, qt * 128 : (qt + 1) * 128],
                            w2_sb[:, di, :],
                            start=(di == 0), stop=(di == 15),
                        )
                for qt in range(3):
                    x = sm.tile([128, D_MODEL], f32, tag="xres")
                    if trivial_b:
                        nc.vector.tensor_tensor(x[:], pf[qt][:], w_sb[:, qt, :], ALU.add)
                    else:
                        nc.vector.scalar_tensor_tensor(
                            x[:], pf[qt][:], 1.0, b2_sb[:], ALU.mult, ALU.add
                        )
                        nc.vector.tensor_tensor(x[:], x[:], w_sb[:, qt, :], ALU.add)
                    _layernorm(
                        nc, sm, w_sb[:, qt, :], x,
                        None if trivial_gb else gb_sb[:, 2, :],
                        None if trivial_gb else gb_sb[:, 3, :],
                        eps_sb[:],
                    )

            for qt in range(3):
                nc.sync.dma_start(
                    out_t[qt * 128 : (qt + 1) * 128, :], w_sb[:, qt, :]
                )

    nc.compile()
    return nc


_NC_CACHE = {}
LAST_RESULT = None


def kernel(**inputs):
    global LAST_RESULT
    trivial_gb = (
        np.all(np.asarray(inputs["ln1_scale"]) == 1.0)
        and np.all(np.asarray(inputs["ln2_scale"]) == 1.0)
        and np.all(np.asarray(inputs["ln1_bias"]) == 0.0)
        and np.all(np.asarray(inputs["ln2_bias"]) == 0.0)
    )
    trivial_b = (
        np.all(np.asarray(inputs["ffn_b1"]) == 0.0)
        and np.all(np.asarray(inputs["ffn_b2"]) == 0.0)
    )
    per_core = _host_prep(inputs)
    drop = []
    if trivial_gb:
        drop.append("gb")
    if trivial_b:
        drop += ["b1col", "b2bc"]
    for pc in per_core:
        for k in drop:
            pc.pop(k, None)
    key = (trivial_gb, trivial_b)
    if key not in _NC_CACHE:
        _NC_CACHE[key] = _build(trivial_gb=trivial_gb, trivial_b=trivial_b)
    res = run_bass_kernel_spmd(
        _NC_CACHE[key], [dict(pc) for pc in per_core], core_ids=list(range(N_CORES)),
        tmpdir=os.environ.get("BASS_TMPDIR") or None,
    )
    LAST_RESULT = res
    out = np.zeros((T, D_MODEL), np.float32)
    for m in range(N_CORES):
        rb, rs = _row_bases(m)
        wout = res.results[m]["wout"]
        out[rb : rb + 128] = wout[0:128]
        out[rs : rs + 128] = wout[128:256]
        if m == 0:
            out[0:32] = wout[256:288]
    return np.ascontiguousarray(out[:, None, :].astype(np.float32))

